# revision 23
# baseline (speedup 1.0000x reference)
"""Mixture-of-Softmaxes Trainium2 kernel (Bass/Tile, 8-core data parallel).

Reference computation (per token t, hidden h[1024]):
  prior  = sigmoid(h @ prior_w + prior_b); prior /= (prior.sum(heads) + 1e-8)
  latent = tanh(h @ latent_w + latent_b).reshape(8, 1024)
  logits = latent @ output_w + output_b                # [8, 2048]
  out    = sum_n prior[n] * softmax(logits[n])         # [2048]

Sharding: data-parallel over the 8192 tokens (B*S), 1024 tokens/core.
All params replicated. Matmul inputs fp16, fp32 PSUM accumulation;
fp16 accumulator/output (host upcasts to fp32).

Device layout (per core, T=1024 tokens), fast (no-bias) path:
  hT   [P, KC, ST] per supertile (kc-major) so phase A starts as soon as
       the first kc chunks + first lw hd-chunk land:
    phase A: latT[hd] = lw[:, hd, kc, :].T @ hT[:, kc, :]  -> [128, ST]
             tanh -> fp16
    phase B: logits = latT_tile.T @ ow_q -> [128 tok, 512] in PSUM
  softmax w/o max-subtract (logits ~ N(0, 0.63^2), exp is safe), denom via
  the ACT accum_out side-output; per-head combine is one fused DVE
  scalar_tensor_tensor: acc = (E * w_n) + acc.

The kernel streams matmuls at the fp16 PE roofline (~213ns per
128x128x512); beyond that the knobs are edge overheads:
  - initial DMAs are chunked (lw0 by hd, hT by kc-pairs, ow by V-quarter)
    so phase A begins after ~0.5MB lands instead of the full 3MB;
  - prior matmuls are emitted after phase A of head 0 (they need all of
    hT0, which lands later than the first chunks);
  - hT1 and the per-head lw prefetch are deferred off the critical window;
  - last head's combine+output DMA run per 512-col quarter to shorten the
    post-matmul tail;
  - N_WARM warmup matmuls bridge the short initial DMA wait and start the
    PE HAM clock ramp.
"""

import os
import numpy as np
import ml_dtypes

B, S, H, NH, V = 4, 2048, 1024, 8, 2048
N_CORES = 8
T = (B * S) // N_CORES          # tokens per core
P = 128
KH = H // P                     # 8 contraction chunks
ST = 512                        # phase-A moving (token) tile
N_ST = T // ST
TT_PER_ST = ST // P
N_TT = T // P
VC = 512                        # logits free-dim chunk (one PSUM bank)
NVC = V // VC
N_WARM = 12                     # PE warmup matmuls (fast path)
HT = ST // 2                    # half-token DMA chunk (hT delivery pacing)
N_WARM_BIAS = 36
EPS = 1e-8

_CACHE = {}


def _build_fast():
    """No-bias fast path."""
    import concourse.bass as bass
    import concourse.mybir as mybir
    import concourse.tile as tile
    from concourse import bacc
    from concourse.bass import ts

    f32 = mybir.dt.float32
    bf16 = mybir.dt.float16  # fp16: same PE rate as bf16, 8x finer mantissa

    KC = KH

    nc = bacc.Bacc("TRN2", target_bir_lowering=False, debug=False)

    # Host pre-arranged layouts (partition dim second so dram[i] matches the
    # SBUF tile exactly; per-partition bytes contiguous for fast streaming).
    hT_d = nc.dram_tensor("hiddenT", [N_ST, 2, P, KC, HT], bf16,
                          kind="ExternalInput")
    pw_d = nc.dram_tensor("prior_w", [P, KC, NH], bf16, kind="ExternalInput")
    lw_d = nc.dram_tensor("latent_w", [NH, P, KH, KC, P], bf16,
                          kind="ExternalInput")
    ow_d = nc.dram_tensor("output_w", [NVC, P, KH, VC], bf16,
                          kind="ExternalInput")
    out_d = nc.dram_tensor("out", [T, V], bf16, kind="ExternalOutput")

    with tile.TileContext(nc) as tc:
        with (
            tc.tile_pool(name="const", bufs=1) as const,
            tc.tile_pool(name="hid", bufs=N_ST) as hpool,
            tc.tile_pool(name="oww", bufs=NVC) as owpool,
            tc.tile_pool(name="pww", bufs=1) as pwpool,
            tc.tile_pool(name="lww", bufs=2) as lwpool,
            tc.tile_pool(name="lat", bufs=2 * KH) as latpool,
            tc.tile_pool(name="ee", bufs=TT_PER_ST + 1) as epool,
            tc.tile_pool(name="acc", bufs=TT_PER_ST + 1) as accpool,
            tc.tile_pool(name="small", bufs=4 * N_TT) as spool,
            tc.tile_pool(name="ps_lat", bufs=3, space="PSUM") as ps_lat,
            tc.tile_pool(name="ps_log", bufs=3, space="PSUM") as ps_log,
            tc.tile_pool(name="ps_pri", bufs=2, space="PSUM") as ps_pri,
        ):
            # ---- PE warmup ------------------------------------------------
            warm = const.tile([P, P + ST], bf16, tag="warm")
            nc.vector.memset(warm[:], 0.0)
            for _ in range(N_WARM):
                wp = ps_lat.tile([P, ST], f32, tag="lat")
                nc.tensor.matmul(wp[:], warm[:, 0:P], warm[:, P:P + ST],
                                 start=True, stop=True)

            # ---- initial DMAs ---------------------------------------------
            # Two hardware queues, each FIFO in transfer order (scalar is the
            # faster one, ~200GB/s vs ~140GB/s; aggregate ~340). Everything
            # is chunked and laid out in consumption order so head-0 compute
            # paces with delivery:
            #   scalar: lw0 chunk 0, six hT0 half-pair chunks, lw0 chunks
            #           1-7, ow q0, then the per-head lw stream
            #   sync:   pw, last two hT0 chunks, ow q1-q3 (later: hT1, outs)
            hTs = []
            for sti in range(N_ST):
                t = hpool.tile([P, 2, KC, HT], bf16, tag="hT")
                hTs.append(t)

            def load_hT(sti, h, j, engine):
                engine.dma_start(
                    hTs[sti][:, h, 2 * j:2 * j + 2, :],
                    hT_d[sti, h, :, 2 * j:2 * j + 2, :])

            def load_lw(n, hds=range(KH), engine=None):
                # NB: engine choice = both the hardware queue the transfer
                # rides AND whose instruction queue pays the ~0.6us issue
                # cost per chunk. Steady-state loads go on sync (idle);
                # only lw0 rides scalar (ahead of the ow quarters).
                t = lwpool.tile([P, KH, KC, P], bf16, tag="lw")
                for hd in hds:
                    (engine or nc.sync).dma_start(t[:, hd], lw_d[n, :, hd])
                return t

            pw = pwpool.tile([P, KC, NH], bf16, tag="pw")
            ows = []
            for q in range(NVC):
                owt = owpool.tile([P, KH, VC], bf16, tag="ow")
                ows.append(owt)

            lw_next = load_lw(0, hds=[0, 1, 2], engine=nc.scalar)
            nc.sync.dma_start(pw[:], pw_d[:])    # sync: pw tiny
            for h, j in [(0, 0), (0, 1), (0, 2), (0, 3), (1, 0), (1, 1)]:
                load_hT(0, h, j, nc.scalar)
            for h, j in [(1, 2), (1, 3)]:
                load_hT(0, h, j, nc.sync)
            for hd in range(3, KH):
                nc.scalar.dma_start(lw_next[:, hd], lw_d[0, :, hd])
            nc.scalar.dma_start(ows[0][:], ow_d[0])
            for q in range(1, NVC):
                nc.sync.dma_start(ows[q][:], ow_d[q])

            # ---- prior: wgt[tt] = sigmoid(h@pw) / (sum + EPS) -------------
            wgt = [None] * N_TT

            def prior_tts(tts):
              for tt in tts:
                pr_ps = ps_pri.tile([P, NH], f32, tag="pri")
                tt2 = tt % TT_PER_ST
                for kc in range(KC):
                    nc.tensor.matmul(
                        pr_ps[:],
                        hTs[tt // TT_PER_ST][:, tt2 // 2, kc,
                                             ts(tt2 % 2, P)],
                        pw[:, kc, :],
                        start=(kc == 0),
                        stop=(kc == KC - 1),
                    )
                sig = spool.tile([P, NH], f32, tag="sig")
                ssum = spool.tile([P, 1], f32, tag="ssum")
                nc.scalar.activation(
                    sig[:], pr_ps[:], mybir.ActivationFunctionType.Sigmoid,
                    accum_out=ssum[:],
                )
                nc.vector.tensor_scalar_add(ssum[:], ssum[:], float(EPS))
                inv = spool.tile([P, 1], f32, tag="inv")
                nc.vector.reciprocal(inv[:], ssum[:])
                w = spool.tile([P, NH], f32, tag="wgt")
                nc.vector.tensor_scalar_mul(w[:], sig[:], inv[:])
                wgt[tt] = w

            # ---- main: per 512-token supertile, per head ------------------
            for st in range(N_ST):
                if st > 0:
                    prior_tts(range(st * TT_PER_ST, (st + 1) * TT_PER_ST))
                acc = {}
                for n in range(NH):
                    # phase A: latT[hd] [128, ST] fp16 = tanh(lw_n.T @ hT_st)
                    lw_n = lw_next
                    latT = []
                    if st == 0 and n == 0:
                        # kc-outer over 3-hd blocks (3 PSUM banks live):
                        # each hT kc-chunk is consumed as it lands instead
                        # of hd0 demanding all of hT0 in one 3us burst —
                        # keeps the PE gap-free during the initial DMA burst
                        # so the HAM clock ramps once and stays warm.
                        for b0 in range(0, KH, 3):
                            bn = min(3, KH - b0)
                            pss = []
                            for _ in range(bn):
                                bps = ps_lat.tile([P, ST], f32, tag="lat")
                                pss.append(bps)
                            for kc in range(KC):
                                for bi in range(bn):
                                    nc.tensor.matmul(
                                        pss[bi][:],
                                        lw_n[:, b0 + bi, kc, :],
                                        hTs[0][:, :, kc, :],
                                        start=(kc == 0),
                                        stop=(kc == KC - 1),
                                    )
                            for bi in range(bn):
                                lt = latpool.tile([P, ST], bf16, tag="latT")
                                nc.scalar.activation(
                                    lt[:], pss[bi][:],
                                    mybir.ActivationFunctionType.Tanh
                                )
                                latT.append(lt)
                    else:
                        for hd in range(KH):
                            lat_ps = ps_lat.tile([P, ST], f32, tag="lat")
                            for kc in range(KC):
                                nc.tensor.matmul(
                                    lat_ps[:],
                                    lw_n[:, hd, kc, :],
                                    hTs[st][:, :, kc, :],
                                    start=(kc == 0),
                                    stop=(kc == KC - 1),
                                )
                            lt = latpool.tile([P, ST], bf16, tag="latT")
                            nc.scalar.activation(
                                lt[:], lat_ps[:],
                                mybir.ActivationFunctionType.Tanh
                            )
                            latT.append(lt)

                    if st == 0 and n == 0:
                        # prior needs all of hT0 — emit after phase A so the
                        # PE isn't blocked on the full 1MB landing.
                        prior_tts(range(TT_PER_ST))
                    if st == 0 and n == 2:
                        # hT1 far off the critical DMA window (needed at st1)
                        for h in range(2):
                            for j in range(KC // 2):
                                load_hT(1, h, j, nc.sync)
                    if not (st == 0 and n == 0) and \
                            not (st == N_ST - 1 and n == NH - 1):
                        lw_next = load_lw((n + 1) % NH)

                    # phase B. Steady state: tt-outer, per V-quarter [P,512]
                    # PSUM; exp(q) overlaps the matmuls of q+1. Head 0 runs
                    # q-OUTER so ow quarter q isn't needed until ~q*3.4us
                    # into phase B — relaxing the initial-burst deadlines.
                    HV = V // 2

                    def combine(tt, E, ds4):
                        dsm = spool.tile([P, 1], f32, tag="dsm")
                        nc.vector.tensor_reduce(
                            dsm[:], ds4[:], axis=mybir.AxisListType.X,
                            op=mybir.AluOpType.add,
                        )
                        invd = spool.tile([P, 1], f32, tag="invd")
                        nc.vector.reciprocal(invd[:], dsm[:])
                        wn = spool.tile([P, 1], f32, tag="wn")
                        nc.vector.tensor_tensor(
                            wn[:], wgt[tt][:, n:n + 1], invd[:],
                            op=mybir.AluOpType.mult,
                        )
                        if n == 0:
                            a = accpool.tile([P, V], bf16, tag="acc")
                            acc[tt % TT_PER_ST] = a
                        else:
                            a = acc[tt % TT_PER_ST]
                        for half in range(2):
                            tgt = a[:, ts(half, HV)]
                            Eh = E[:, ts(half, HV)]
                            if n == 0:
                                nc.vector.tensor_scalar_mul(tgt, Eh, wn[:])
                            else:
                                nc.vector.scalar_tensor_tensor(
                                    tgt, Eh, wn[:], tgt,
                                    op0=mybir.AluOpType.mult,
                                    op1=mybir.AluOpType.add,
                                )
                            if n == NH - 1:
                                nc.sync.dma_start(
                                    out_d[ts(tt, P), ts(half, HV)], tgt
                                )

                    def logits_exp(tti, q, E, ds4):
                        lg_ps = ps_log.tile([P, VC], f32, tag="log")
                        for hd in range(KH):
                            nc.tensor.matmul(
                                lg_ps[:],
                                latT[hd][:, ts(tti, P)],
                                ows[q][:, hd, :],
                                start=(hd == 0),
                                stop=(hd == KH - 1),
                            )
                        nc.scalar.activation(
                            E[:, ts(q, VC)], lg_ps[:],
                            mybir.ActivationFunctionType.Exp,
                            accum_out=ds4[:, q:q + 1],
                        )

                    if st == 0 and n == 0:
                        Es, dss = [], []
                        for _ in range(TT_PER_ST):
                            Et = epool.tile([P, V], bf16, tag="E")
                            dst = spool.tile([P, NVC], f32, tag="ds4")
                            Es.append(Et)
                            dss.append(dst)
                        for q in range(NVC):
                            for tti in range(TT_PER_ST):
                                logits_exp(tti, q, Es[tti], dss[tti])
                            if q == 0:
                                # lw1 queued behind everything critical
                                lw_next = load_lw(1)
                        for tti in range(TT_PER_ST):
                            combine(tti, Es[tti], dss[tti])
                    else:
                        for tti in range(TT_PER_ST):
                            tt = st * TT_PER_ST + tti
                            E = epool.tile([P, V], bf16, tag="E")
                            ds4 = spool.tile([P, NVC], f32, tag="ds4")
                            for q in range(NVC):
                                logits_exp(tti, q, E, ds4)
                            combine(tt, E, ds4)

    nc.compile()
    return nc


def _build_bias():
    """Original (baseline) build, kept for the with-bias fallback."""
    import concourse.bass as bass
    import concourse.mybir as mybir
    import concourse.tile as tile
    from concourse import bacc
    from concourse.bass import ts

    f32 = mybir.dt.float32
    bf16 = mybir.dt.float16
    with_bias = True

    KC = KH + 1
    HD = KH + 1

    nc = bacc.Bacc("TRN2", target_bir_lowering=False, debug=False)

    hT_d = nc.dram_tensor("hiddenT", [N_ST, 4, P, KC, ST // 4], bf16,
                          kind="ExternalInput")
    pw_d = nc.dram_tensor("prior_w", [P, KC, NH], bf16, kind="ExternalInput")
    lw_d = nc.dram_tensor("latent_w", [NH, P, KC, H], bf16,
                          kind="ExternalInput")
    ow_d = nc.dram_tensor("output_w", [P, HD, V], bf16, kind="ExternalInput")
    out_d = nc.dram_tensor("out", [T, V], bf16, kind="ExternalOutput")

    with tile.TileContext(nc) as tc:
        with (
            tc.tile_pool(name="const", bufs=1) as const,
            tc.tile_pool(name="hid", bufs=N_ST) as hpool,
            tc.tile_pool(name="oww", bufs=1) as owpool,
            tc.tile_pool(name="pww", bufs=1) as pwpool,
            tc.tile_pool(name="lww", bufs=2) as lwpool,
            tc.tile_pool(name="lat", bufs=2 * KH) as latpool,
            tc.tile_pool(name="ee", bufs=3) as epool,
            tc.tile_pool(name="acc", bufs=TT_PER_ST + 1) as accpool,
            tc.tile_pool(name="small", bufs=4 * N_TT) as spool,
            tc.tile_pool(name="ps_lat", bufs=3, space="PSUM") as ps_lat,
            tc.tile_pool(name="ps_log", bufs=3, space="PSUM") as ps_log,
            tc.tile_pool(name="ps_pri", bufs=2, space="PSUM") as ps_pri,
        ):
            warm = const.tile([P, P + ST], bf16, tag="warm")
            nc.vector.memset(warm[:], 0.0)
            for _ in range(N_WARM_BIAS):
                wp = ps_lat.tile([P, ST], f32, tag="lat")
                nc.tensor.matmul(wp[:], warm[:, 0:P], warm[:, P:P + ST],
                                 start=True, stop=True)

            hTs = []
            for sti in range(N_ST):
                t = hpool.tile([P, 4, KC, ST // 4], bf16, tag="hT")
                hTs.append(t)

            def load_lw(n, engine=None):
                t = lwpool.tile([P, KC, H], bf16, tag="lw")
                (engine or nc.sync).dma_start(t[:], lw_d[n])
                return t

            nc.sync.dma_start(hTs[0][:, 0], hT_d[0, 0])
            pw = pwpool.tile([P, KC, NH], bf16, tag="pw")
            nc.sync.dma_start(pw[:], pw_d[:])
            for qq in range(1, 4):
                nc.sync.dma_start(hTs[0][:, qq], hT_d[0, qq])
            lw_next = load_lw(0, engine=nc.scalar)
            for qq in range(4):
                nc.sync.dma_start(hTs[1][:, qq], hT_d[1, qq])
            ow = owpool.tile([P, HD, V], bf16, tag="ow")
            nc.scalar.dma_start(ow[:], ow_d[:])
            ones_t = const.tile([P, P], bf16)
            nc.vector.memset(ones_t[:], 0.0)
            nc.vector.memset(ones_t[0:1, :], 1.0)

            wgt = [None] * N_TT

            def prior_for(sti):
              for tt in range(sti * TT_PER_ST, (sti + 1) * TT_PER_ST):
                pr_ps = ps_pri.tile([P, NH], f32, tag="pri")
                for kc in range(KC):
                    nc.tensor.matmul(
                        pr_ps[:],
                        hTs[tt // TT_PER_ST][:, tt % TT_PER_ST, kc, :],
                        pw[:, kc, :],
                        start=(kc == 0),
                        stop=(kc == KC - 1),
                    )
                sig = spool.tile([P, NH], f32, tag="sig")
                ssum = spool.tile([P, 1], f32, tag="ssum")
                nc.scalar.activation(
                    sig[:], pr_ps[:], mybir.ActivationFunctionType.Sigmoid,
                    accum_out=ssum[:],
                )
                nc.vector.tensor_scalar_add(ssum[:], ssum[:], float(EPS))
                inv = spool.tile([P, 1], f32, tag="inv")
                nc.vector.reciprocal(inv[:], ssum[:])
                w = spool.tile([P, NH], f32, tag="wgt")
                nc.vector.tensor_scalar_mul(w[:], sig[:], inv[:])
                wgt[tt] = w

            for st in range(N_ST):
                prior_for(st)
                acc = {}
                for n in range(NH):
                    lw_n = lw_next
                    if not (st == N_ST - 1 and n == NH - 1):
                        lw_next = load_lw((n + 1) % NH)
                    latT = []
                    for hd in range(KH):
                        lat_ps = ps_lat.tile([P, ST], f32, tag="lat")
                        for kc in range(KC):
                            nc.tensor.matmul(
                                lat_ps[:],
                                lw_n[:, kc, ts(hd, P)],
                                hTs[st][:, :, kc, :],
                                start=(kc == 0),
                                stop=(kc == KC - 1),
                            )
                        lt = latpool.tile([P, ST], bf16, tag="latT")
                        nc.scalar.activation(
                            lt[:], lat_ps[:], mybir.ActivationFunctionType.Tanh
                        )
                        latT.append(lt)

                    HV = V // 2
                    for tti in range(TT_PER_ST):
                        tt = st * TT_PER_ST + tti
                        E = epool.tile([P, V], bf16, tag="E")
                        ds4 = spool.tile([P, NVC], f32, tag="ds4")
                        for q in range(NVC):
                            lg_ps = ps_log.tile([P, VC], f32, tag="log")
                            for hd in range(HD):
                                lhsT = (
                                    latT[hd][:, ts(tti, P)]
                                    if hd < KH
                                    else ones_t[:]
                                )
                                nc.tensor.matmul(
                                    lg_ps[:],
                                    lhsT,
                                    ow[:, hd, ts(q, VC)],
                                    start=(hd == 0),
                                    stop=(hd == HD - 1),
                                )
                            nc.scalar.activation(
                                E[:, ts(q, VC)], lg_ps[:],
                                mybir.ActivationFunctionType.Exp,
                                accum_out=ds4[:, q:q + 1],
                            )
                        dsm = spool.tile([P, 1], f32, tag="dsm")
                        nc.vector.tensor_reduce(
                            dsm[:], ds4[:], axis=mybir.AxisListType.X,
                            op=mybir.AluOpType.add,
                        )
                        invd = spool.tile([P, 1], f32, tag="invd")
                        nc.vector.reciprocal(invd[:], dsm[:])
                        wn = spool.tile([P, 1], f32, tag="wn")
                        nc.vector.tensor_tensor(
                            wn[:], wgt[tt][:, n:n + 1], invd[:],
                            op=mybir.AluOpType.mult,
                        )
                        if n == 0:
                            a = accpool.tile([P, V], bf16, tag="acc")
                            acc[tti] = a
                        else:
                            a = acc[tti]
                        for half in range(2):
                            tgt = a[:, ts(half, HV)]
                            Eh = E[:, ts(half, HV)]
                            if n == 0:
                                nc.vector.tensor_scalar_mul(tgt, Eh, wn[:])
                            else:
                                nc.vector.scalar_tensor_tensor(
                                    tgt, Eh, wn[:], tgt,
                                    op0=mybir.AluOpType.mult,
                                    op1=mybir.AluOpType.add,
                                )
                            if n == NH - 1:
                                nc.sync.dma_start(
                                    out_d[ts(tt, P), ts(half, HV)], tgt
                                )

    nc.compile()
    return nc


def _prep_inputs_fast(hidden, prior_w, latent_w, output_w):
    """Device layouts, fast path:
      hiddenT  [P, KC, B*S]-equivalent stored [N_ST, P, KC, ST] per core
      prior_w  [P, KC, NH]
      latent_w [NH, P, KH, KC, 128]
      output_w [NVC, P, KH, VC]
    """
    bf16 = np.float16
    KC = KH
    BS = B * S

    h = hidden.reshape(BS, H).astype(bf16)
    pw_dev = np.ascontiguousarray(
        prior_w.astype(bf16).reshape(KC, P, NH).transpose(1, 0, 2))
    lw_dev = np.ascontiguousarray(
        latent_w.astype(bf16).reshape(KC, P, NH, KH, P)
        .transpose(2, 1, 3, 0, 4))                       # [NH,P,KH,KC,128]
    ow_dev = np.ascontiguousarray(
        output_w.astype(bf16).reshape(KH, P, NVC, VC)
        .transpose(2, 1, 0, 3))                          # [NVC,P,KH,VC]
    return h, pw_dev, lw_dev, ow_dev


def _prep_inputs_bias(hidden, prior_w, prior_b, latent_w, latent_b, output_w,
                      output_b):
    bf16 = np.float16
    KC = KH + 1
    HD = KH + 1
    BS = B * S

    h = hidden.reshape(BS, H).astype(bf16)
    pw = prior_w.astype(bf16)
    lw = latent_w.astype(bf16)
    ow = output_w.astype(bf16)
    hx = np.zeros((BS, P), bf16)
    hx[:, 0] = 1.0
    h = np.concatenate([h, hx], axis=1)                   # [BS, KC*P]
    pw = np.concatenate(
        [pw, prior_b.astype(bf16)[None, :], np.zeros((P - 1, NH), bf16)],
        axis=0)
    lw = np.concatenate(
        [lw, latent_b.astype(bf16)[None, :],
         np.zeros((P - 1, NH * H), bf16)], axis=0)
    ow = np.concatenate(
        [ow, output_b.astype(bf16)[None, :], np.zeros((P - 1, V), bf16)],
        axis=0)

    hT = h.reshape(BS, KC, P).transpose(2, 1, 0)          # [P, KC, BS]
    pw_dev = np.ascontiguousarray(pw.reshape(KC, P, NH).transpose(1, 0, 2))
    lw_dev = np.ascontiguousarray(
        lw.reshape(KC, P, NH, H).transpose(2, 1, 0, 3))   # [NH, P, KC, H]
    ow_dev = np.ascontiguousarray(ow.reshape(HD, P, V).transpose(1, 0, 2))
    return hT, pw_dev, lw_dev, ow_dev


def kernel(hidden, prior_w, prior_b, latent_w, latent_b, output_w, output_b,
           _profile=False):
    from concourse.bass_utils import run_bass_kernel_spmd

    # coerce to host numpy (the caller may hand us jax arrays)
    hidden = np.asarray(hidden, dtype=np.float32)
    prior_w = np.asarray(prior_w, dtype=np.float32)
    prior_b = np.asarray(prior_b, dtype=np.float32)
    latent_w = np.asarray(latent_w, dtype=np.float32)
    latent_b = np.asarray(latent_b, dtype=np.float32)
    output_w = np.asarray(output_w, dtype=np.float32)
    output_b = np.asarray(output_b, dtype=np.float32)

    with_bias = bool(
        np.any(prior_b) or np.any(latent_b) or np.any(output_b)
    )
    key = with_bias
    if key not in _CACHE:
        _CACHE[key] = _build_bias() if with_bias else _build_fast()
    nc = _CACHE[key]

    in_maps = []
    if with_bias:
        hT, pw, lw, ow = _prep_inputs_bias(
            hidden, prior_w, prior_b, latent_w, latent_b, output_w, output_b)
        for c in range(N_CORES):
            in_maps.append({
                "hiddenT": np.stack(
                    [hT[:, :,
                        c * T + hf * (ST // 4): c * T + (hf + 1) * (ST // 4)]
                     for hf in range(4 * N_ST)]).reshape(
                         N_ST, 4, P, hT.shape[1], ST // 4),
                "prior_w": pw,
                "latent_w": lw,
                "output_w": ow,
            })
    else:
        h, pw, lw, ow = _prep_inputs_fast(hidden, prior_w, latent_w, output_w)
        for c in range(N_CORES):
            hc = h[c * T:(c + 1) * T]                     # [T, KC*P]
            # [N_ST, 2, P, KC, HT]: token t = st*ST + h*HT + j,
            # column k = kc*P + p
            hdev = np.ascontiguousarray(
                hc.reshape(N_ST, 2, HT, KH, P).transpose(0, 1, 4, 3, 2))
            in_maps.append({
                "hiddenT": hdev,
                "prior_w": pw,
                "latent_w": lw,
                "output_w": ow,
            })

    res = run_bass_kernel_spmd(
        nc, in_maps, list(range(N_CORES)), trace=_profile
    )
    out = np.concatenate([res.results[c]["out"] for c in range(N_CORES)],
                         axis=0).astype(np.float32)
    if _profile:
        kernel.last_result = res
    return out.reshape(B, S, V)
